# revision 21
# baseline (speedup 1.0000x reference)
"""Trainium2 Bass kernel for the differentiable gaussian-splat renderer.

Full-input contract: kernel(**inputs) takes the unsharded inputs and returns
the full [2*16, 3, 32, 32] output.

Math (per pose):
    cam = positions @ R.T + t ;  pj = (fx*cam_x/cam_z + cx, fy*cam_y/cam_z + cy)
    w[n, p] = op_n * exp(-0.5*((px-ax_n)^2 + (py-ay_n)^2)/s_n^2)
    img = (w.T @ colors) / (w.T @ 1 + 1e-8)

The gaussian weight is separable: w = wx[n,px] * wy[n,py] (opacity folded into
wx via ln(op) added to the constant coefficient), so instead of N*HW
exponentials we need N*(W + H) and the pixel accumulation becomes a
K=128-chunked matmul  out[py, (c,px)] += wy_chunk.T @ (car_chunk (*) wx_chunk).

Sharding: 8 independent cores = 2 poses x 4 px-column blocks (32 px each).
No collectives; each core computes all 4096 gaussians for its (pose, px-block)
and writes a [128, 96] slab = (py, 32c+px_local). Host reassembles.

All O(N) per-gaussian geometry (projection, quadratic coefficients
[g, -2*g*ax', g*ax'^2 + ln(op)], their exact 3-way bf16 splits, and the
transposed coefficient packs the PE consumes) is precomputed on host; the
device does the O(N*(W+H)) separable evaluation (arg matmuls + exp) and the
O(N*H*W/8) accumulation. The color array is pre-expanded on host to
car[p, (j,c,px)] so the X-build is a contiguous bf16 multiply (DVE fast
path) instead of a slow stride-0-inner broadcast.
"""

import numpy as np

H = 128
W = 128
FX = 120.0
FY = 120.0
CX = 64.0
CY = 64.0
N = 4096
NCHUNK = 32          # 4096 / 128
NPOSE = 2
PXB = 32             # px columns per core
NBLK = 4             # px blocks
NG = 4               # 8-chunk groups
F32 = np.float32

_CACHE = {}


def _quat2mat(q):
    q = np.asarray(q, dtype=np.float64)
    q = q / np.linalg.norm(q)
    w, x, y, z = q
    return np.array([
        [1 - 2 * (y * y + z * z), 2 * (x * y - z * w), 2 * (x * z + y * w)],
        [2 * (x * y + z * w), 1 - 2 * (x * x + z * z), 2 * (y * z - x * w)],
        [2 * (x * z - y * w), 2 * (y * z + x * w), 1 - 2 * (x * x + y * y)],
    ])


def _build_program():
    """Build the SPMD Bass/Tile program (same program on every core)."""
    import concourse.bacc as bacc
    import concourse.tile as tile
    import concourse.mybir as mybir
    from contextlib import ExitStack

    dt = mybir.dt.float32
    bf = mybir.dt.bfloat16
    nc = bacc.Bacc()

    # ---- DRAM I/O (per-core shapes) ----
    # basA (bf16): cols 0:256 basis_x | 256:768 coefT_x (4 tiles of [128,128])
    basA_d = nc.dram_tensor("basA", [128, 768], bf, kind="ExternalInput").ap()
    # basB (bf16): cols 0:128 coefT_y tile0 | 128:1152 basis_y8 |
    #              1152:1536 coefT_y tiles 1-3
    basB_d = nc.dram_tensor("basB", [128, 1536], bf, kind="ExternalInput").ap()
    # car (bf16): col 1024*g + 128*jrel + 32*c + px = color_c(gauss) (c=3: 1.0)
    car_d = [nc.dram_tensor(f"car{h}", [128, 2048], bf, kind="ExternalInput").ap()
             for h in range(2)]
    out_d = nc.dram_tensor("out", [128, 96], dt, kind="ExternalOutput").ap()
    warm_d = nc.dram_tensor("warm", [1, 64], bf, kind="Internal").ap()

    add = mybir.AluOpType.add
    EXP = mybir.ActivationFunctionType.Exp

    with tile.TileContext(nc) as tc, ExitStack() as ctx:
        const = ctx.enter_context(tc.tile_pool(name="const", bufs=1))
        psum_arg = ctx.enter_context(tc.tile_pool(name="psum_arg", bufs=3, space="PSUM"))
        psum_out = ctx.enter_context(tc.tile_pool(name="psum_out", bufs=1, space="PSUM"))

        po = psum_out.tile([128, 128], dt, tag="po")  # claim psum bank 0 first

        basA = const.tile([128, 768], bf, tag="basA")
        nc.sync.dma_start(out=basA[:], in_=basA_d)
        basB = const.tile([128, 1536], bf, tag="basB")
        nc.sync.dma_start(out=basB[:], in_=basB_d)
        car = const.tile([128, 4096], bf, tag="car")
        for h in range(2):
            nc.sync.dma_start(out=car[:, 2048 * h:2048 * h + 2048], in_=car_d[h])

        basis_x = basA[:, 0:256]

        def ctx_tile(t):
            return basA[:, 256 + 128 * t:384 + 128 * t]

        def cty_tile(g):
            return basB[:, 0:128] if g == 0 else basB[:, 1024 + 128 * g:1152 + 128 * g]

        basis_y8 = basB[:, 128:1152]

        # ---- wx args: 4 matmuls [128,128]x[128,256] -> one exp -> bf16 ----
        pa_x = psum_arg.tile([128, 1024], dt, tag="pa")
        for t in range(4):
            nc.tensor.matmul(pa_x[:, 256 * t:256 * t + 256],
                             lhsT=ctx_tile(t),
                             rhs=basis_x, start=True, stop=True)
        w_x = const.tile([128, 1024], bf, tag="wx")
        nc.scalar.activation(out=w_x[:], in_=pa_x[:], func=EXP)

        # ---- X build: per 8-chunk group, X = car (*) wx  (all bf16,
        #      inner-dim contiguous -> DVE fast path) ----
        xs = []
        for g in range(NG):
            Xg = const.tile([128, 1024], bf, tag=f"X{g}")
            car_v = car[:, 1024 * g:1024 * g + 1024].rearrange(
                "p (j c x) -> p j c x", j=8, c=4)
            wx_v = w_x[:, 256 * g:256 * g + 256].rearrange(
                "p (j x) -> p j x", j=8).unsqueeze(2).broadcast_to([128, 8, 4, 32])
            out_v = Xg[:].rearrange("p (j c x) -> p j c x", j=8, c=4)
            nc.vector.tensor_mul(out=out_v, in0=car_v, in1=wx_v)
            xs.append(Xg)

        # ---- wy args (2 matmuls per group) + exp: all emitted upfront so
        #      no acc batch sits between a wy matmul and its exp (the sem
        #      update otherwise lands late and stalls the scalar engine) ----
        wys = []
        for g in range(NG):
            pa = psum_arg.tile([128, 1024], dt, tag="pa")
            for h in range(2):
                nc.tensor.matmul(pa[:, 512 * h:512 * h + 512],
                                 lhsT=cty_tile(g),
                                 rhs=basis_y8[:, 512 * h:512 * h + 512],
                                 start=True, stop=True)
            wt = const.tile([128, 1024], bf, tag=f"wy{g}")
            nc.scalar.activation(out=wt[:], in_=pa[:], func=EXP)
            wys.append(wt)

        for g in range(NG):
            wyt = wys[g]
            for jrel in range(8):
                j = 8 * g + jrel
                nc.tensor.matmul(po[:],
                                 lhsT=wyt[:, 128 * jrel:128 * jrel + 128],
                                 rhs=xs[g][:, 128 * jrel:128 * jrel + 128],
                                 start=(j == 0), stop=(j == NCHUNK - 1))

        # ---- normalize: img = num * (1/(den + 1e-8)) ----
        dent = const.tile([128, 32], dt, tag="dent")
        nc.vector.tensor_scalar(out=dent[:], in0=po[:, 96:128], scalar1=1e-8,
                                scalar2=None, op0=add)
        dr = const.tile([128, 32], dt, tag="dr")
        nc.vector.reciprocal_approx_fast(out=dr[:], in_=dent[:])
        img = const.tile([128, 96], dt, tag="img")
        img_r = img[:].rearrange("p (c x) -> p c x", c=3)
        num_r = po[:, 0:96].rearrange("p (c x) -> p c x", c=3)
        dr_b = dr[:].unsqueeze(1).broadcast_to([128, 3, 32])
        nc.vector.tensor_mul(out=img_r, in0=num_r, in1=dr_b)
        nc.gpsimd.dma_start(out=out_d, in_=img[:])

    nc.compile()   # legalizes sync waits (HW allows 1/instruction) etc.
    return nc


def _split3(v, bf):
    """Exact-ish 3-way bf16 split of float64/float32 array v."""
    v = v.astype(F32)
    p1 = v.astype(bf)
    r1 = (v - p1.astype(F32)).astype(F32)
    p2 = r1.astype(bf)
    r2 = (r1 - p2.astype(F32)).astype(F32)
    p3 = r2.astype(bf)
    return p1, p2, p3


def _host_prep(positions, colors, opacities, scales, qvec, tvec):
    """Build the 8 per-core input maps (all O(N) numpy work)."""
    import ml_dtypes
    bf = ml_dtypes.bfloat16

    positions = np.asarray(positions, dtype=np.float64)
    colors = np.asarray(colors, dtype=F32)
    opacities = np.asarray(opacities, dtype=np.float64)
    scales = np.asarray(scales, dtype=np.float64)
    qvec = np.asarray(qvec, dtype=F32)
    tvec = np.asarray(tvec, dtype=F32)

    g_coef = -0.5 / (scales[:, 0] ** 2)                       # [N]
    lnop = np.log(np.maximum(opacities[:, 0], 1e-300))        # [N]

    def basis_rows(q):
        """[16, len(q)] bf16 rows: p2h,p2l,p2h,p2l,p2h,p2l,q,q,q,1,1,1,0*4."""
        q = q.astype(F32)
        p2 = (q * q).astype(F32)
        p2h = p2.astype(bf)
        p2l = (p2 - p2h.astype(F32)).astype(F32).astype(bf)
        qb = q.astype(bf)
        one = np.ones_like(q, dtype=bf)
        zero = np.zeros_like(q, dtype=bf)
        return np.stack([p2h, p2l, p2h, p2l, p2h, p2l,
                         qb, qb, qb, one, one, one, zero, zero, zero, zero])

    def coefT(A, B, C):
        """[N] coefs -> [128, 512] bf16 transposed pack.

        Tile t (cols 128t:128t+128): row 16*jrel + r, col = n_in_chunk,
        rows r: (A1,A1,A2,A2,A3,A3,B1,B2,B3,C1,C2,C3,0,0,0,0)."""
        a1, a2, a3 = _split3(A, bf)
        b1, b2, b3 = _split3(B, bf)
        c1, c2, c3 = _split3(C, bf)
        zero = np.zeros_like(a1)
        rows = np.stack([a1, a1, a2, a2, a3, a3, b1, b2, b3, c1, c2, c3,
                         zero, zero, zero, zero])            # [16, N]
        # [16, 32 chunks, 128 n] -> per tile t: [16*8jrel rows, 128]
        rows = rows.reshape(16, NCHUNK, 128)
        pack = np.zeros((128, 512), bf)
        for j in range(NCHUNK):
            t, jrel = j // 8, j % 8
            pack[16 * jrel:16 * jrel + 16, 128 * t:128 * t + 128] = rows[:, j, :]
        return pack

    # basis_y8 [128, 1024]: block-diag, rows 16*jrel+r, cols 128*jrel + py
    py = np.arange(128) - CY
    by_rows = basis_rows(py)                      # [16, 128]
    basis_y8 = np.zeros((128, 1024), bf)
    for jrel in range(8):
        basis_y8[16 * jrel:16 * jrel + 16, 128 * jrel:128 * jrel + 128] = by_rows

    # car [128, 4096] bf16: col 128*j + 32*c + px = color_c(128j+p) (c=3: 1)
    colc = np.concatenate([colors, np.ones((N, 1), F32)], axis=1)  # [N, 4]
    car = colc.reshape(NCHUNK, 128, 4).transpose(1, 0, 2)          # [128, j, c]
    car = np.repeat(car[:, :, :, None], PXB, axis=3)               # [128,j,c,px]
    car = np.ascontiguousarray(car.reshape(128, NCHUNK * 128)).astype(bf)

    in_maps = []
    for p in range(NPOSE):
        R = _quat2mat(qvec[p])
        t64 = tvec[p].astype(np.float64)
        u = positions @ (FX * R[0]) + FX * t64[0]
        v = positions @ (FY * R[1]) + FY * t64[1]
        zc = positions @ R[2] + t64[2]
        ax = u / zc + CX          # absolute px coords of gaussian center
        ay = v / zc + CY
        ayc = ay - CY             # y centered at 64
        coefT_y = coefT(g_coef, -2.0 * g_coef * ayc, g_coef * ayc * ayc)
        basB = np.zeros((128, 1536), bf)
        basB[:, 0:128] = coefT_y[:, 0:128]
        basB[:, 128:1152] = basis_y8
        basB[:, 1152:1536] = coefT_y[:, 128:512]

        for b in range(NBLK):
            cb = 32.0 * b + 16.0                  # block center
            axc = ax - cb
            coefT_x = coefT(g_coef, -2.0 * g_coef * axc,
                            g_coef * axc * axc + lnop)
            px = np.arange(PXB * b, PXB * b + PXB) - cb   # in [-16, 16)
            bx_rows = basis_rows(px)                      # [16, 32]
            basis_x = np.zeros((128, 256), bf)
            for jrel in range(8):
                basis_x[16 * jrel:16 * jrel + 16,
                        32 * jrel:32 * jrel + 32] = bx_rows
            basA = np.zeros((128, 768), bf)
            basA[:, 0:256] = basis_x
            basA[:, 256:768] = coefT_x
            m = {"basA": basA, "basB": basB}
            for h in range(2):
                m[f"car{h}"] = np.ascontiguousarray(
                    car[:, 2048 * h:2048 * h + 2048])
            in_maps.append(m)
    return in_maps


def _assemble(slabs):
    """slabs: list of 8 [128, 96] arrays -> [NPOSE*16, 3, 32, 32] output."""
    out = []
    for p in range(NPOSE):
        img = np.zeros((H, W, 3), F32)
        for b in range(NBLK):
            slab = slabs[p * NBLK + b]
            for c in range(3):
                img[:, PXB * b:PXB * b + PXB, c] = slab[:, 32 * c:32 * c + 32]
        tiles = img.reshape(H * W, 3).reshape(16, 1024, 3)
        tiles = tiles.transpose(0, 2, 1).reshape(16, 3, 32, 32)
        out.append(tiles)
    return np.concatenate(out, axis=0).astype(F32)


def kernel(positions, colors, opacities, scales, qvec, tvec, _trace=False):
    from concourse.bass_utils import run_bass_kernel_spmd

    if "nc" not in _CACHE:
        _CACHE["nc"] = _build_program()
    nc = _CACHE["nc"]

    in_maps = _host_prep(positions, colors, opacities, scales, qvec, tvec)
    res = run_bass_kernel_spmd(nc, in_maps, core_ids=list(range(8)),
                               trace=_trace)
    slabs = [np.asarray(res.results[c]["out"]) for c in range(8)]
    out = _assemble(slabs)
    if _trace:
        _CACHE["last_result"] = res
    return out


# revision 22
# speedup vs baseline: 1.1891x; 1.1891x over previous
"""Trainium2 Bass kernel for the differentiable gaussian-splat renderer.

Full-input contract: kernel(**inputs) takes the unsharded inputs and returns
the full [2*16, 3, 32, 32] output.

Math (per pose):
    cam = positions @ R.T + t ;  pj = (fx*cam_x/cam_z + cx, fy*cam_y/cam_z + cy)
    w[n, p] = op_n * exp(-0.5*((px-ax_n)^2 + (py-ay_n)^2)/s_n^2)
    img = (w.T @ colors) / (w.T @ 1 + 1e-8)

The gaussian weight is separable: w = wx[n,px] * wy[n,py] (opacity folded into
wx via ln(op) added to the constant coefficient), so instead of N*HW
exponentials we need N*(W + H) and the pixel accumulation becomes a
K=128-chunked matmul  out[py, (c,px)] += wy_chunk.T @ (car_chunk (*) wx_chunk).

Sharding: 8 independent cores = 2 poses x 4 px-column blocks (32 px each).
No collectives; each core computes all 4096 gaussians for its (pose, px-block)
and writes a [128, 96] slab = (py, 32c+px_local). Host reassembles.

All O(N) per-gaussian geometry (projection, quadratic coefficients
[g, -2*g*ax', g*ax'^2 + ln(op)], their exact 3-way bf16 splits, and the
transposed coefficient packs the PE consumes) is precomputed on host; the
device does the O(N*(W+H)) separable evaluation (arg matmuls + exp) and the
O(N*H*W/8) accumulation. The color array is pre-expanded on host to
car[p, (j,c,px)] so the X-build is a contiguous bf16 multiply (DVE fast
path) instead of a slow stride-0-inner broadcast.
"""

import numpy as np

H = 128
W = 128
FX = 120.0
FY = 120.0
CX = 64.0
CY = 64.0
N = 4096
NCHUNK = 32          # 4096 / 128
NPOSE = 2
PXB = 32             # px columns per core
NBLK = 4             # px blocks
NG = 4               # 8-chunk groups
F32 = np.float32

_CACHE = {}


def _quat2mat(q):
    q = np.asarray(q, dtype=np.float64)
    q = q / np.linalg.norm(q)
    w, x, y, z = q
    return np.array([
        [1 - 2 * (y * y + z * z), 2 * (x * y - z * w), 2 * (x * z + y * w)],
        [2 * (x * y + z * w), 1 - 2 * (x * x + z * z), 2 * (y * z - x * w)],
        [2 * (x * z - y * w), 2 * (y * z + x * w), 1 - 2 * (x * x + y * y)],
    ])


def _build_program():
    """Build the SPMD Bass/Tile program (same program on every core)."""
    import concourse.bacc as bacc
    import concourse.tile as tile
    import concourse.mybir as mybir
    from contextlib import ExitStack

    dt = mybir.dt.float32
    bf = mybir.dt.bfloat16
    nc = bacc.Bacc()

    # ---- DRAM I/O (per-core shapes) ----
    # basA (bf16): cols 0:256 basis_x | 256:768 coefT_x (4 tiles of [128,128])
    basA_d = nc.dram_tensor("basA", [128, 768], bf, kind="ExternalInput").ap()
    # basB (bf16): cols 0:128 coefT_y tile0 | 128:1152 basis_y8 |
    #              1152:1536 coefT_y tiles 1-3
    basB_d = nc.dram_tensor("basB", [128, 1536], bf, kind="ExternalInput").ap()
    # car (bf16): col 1024*g + 128*jrel + 32*c + px = color_c(gauss) (c=3: 1.0)
    car_d = [nc.dram_tensor(f"car{h}", [128, 2048], bf, kind="ExternalInput").ap()
             for h in range(2)]
    out_d = nc.dram_tensor("out", [128, 96], dt, kind="ExternalOutput").ap()
    warm_d = nc.dram_tensor("warm", [1, 64], bf, kind="Internal").ap()

    add = mybir.AluOpType.add
    EXP = mybir.ActivationFunctionType.Exp

    with tile.TileContext(nc) as tc, ExitStack() as ctx:
        const = ctx.enter_context(tc.tile_pool(name="const", bufs=1))
        psum_arg = ctx.enter_context(tc.tile_pool(name="psum_arg", bufs=3, space="PSUM"))
        psum_out = ctx.enter_context(tc.tile_pool(name="psum_out", bufs=1, space="PSUM"))

        po = psum_out.tile([128, 128], dt, tag="po")  # claim psum bank 0 first

        basA = const.tile([128, 768], bf, tag="basA")
        nc.sync.dma_start(out=basA[:], in_=basA_d)
        basB = const.tile([128, 1536], bf, tag="basB")
        nc.sync.dma_start(out=basB[:], in_=basB_d)
        car = const.tile([128, 4096], bf, tag="car")
        for h in range(2):
            nc.sync.dma_start(out=car[:, 2048 * h:2048 * h + 2048], in_=car_d[h])

        basis_x = basA[:, 0:256]

        def ctx_tile(t):
            return basA[:, 256 + 128 * t:384 + 128 * t]

        def cty_tile(g):
            return basB[:, 0:128] if g == 0 else basB[:, 1024 + 128 * g:1152 + 128 * g]

        basis_y8 = basB[:, 128:1152]

        # ---- wx args: 4 matmuls [128,128]x[128,256] -> one exp -> bf16 ----
        pa_x = psum_arg.tile([128, 1024], dt, tag="pa")
        for t in range(4):
            nc.tensor.matmul(pa_x[:, 256 * t:256 * t + 256],
                             lhsT=ctx_tile(t),
                             rhs=basis_x, start=True, stop=True)
        w_x = const.tile([128, 1024], bf, tag="wx")
        nc.scalar.activation(out=w_x[:], in_=pa_x[:], func=EXP)

        # ---- X build: per 8-chunk group, X = car (*) wx  (all bf16,
        #      inner-dim contiguous -> DVE fast path) ----
        xs = []
        for g in range(NG):
            Xg = const.tile([128, 1024], bf, tag=f"X{g}")
            car_v = car[:, 1024 * g:1024 * g + 1024].rearrange(
                "p (j c x) -> p j c x", j=8, c=4)
            wx_v = w_x[:, 256 * g:256 * g + 256].rearrange(
                "p (j x) -> p j x", j=8).unsqueeze(2).broadcast_to([128, 8, 4, 32])
            out_v = Xg[:].rearrange("p (j c x) -> p j c x", j=8, c=4)
            nc.vector.tensor_mul(out=out_v, in0=car_v, in1=wx_v)
            xs.append(Xg)

        # ---- wy args (2 matmuls per group) + exp: all emitted upfront so
        #      no acc batch sits between a wy matmul and its exp (the sem
        #      update otherwise lands late and stalls the scalar engine) ----
        wys = []
        for g in range(NG):
            pa = psum_arg.tile([128, 1024], dt, tag="pa")
            for h in range(2):
                nc.tensor.matmul(pa[:, 512 * h:512 * h + 512],
                                 lhsT=cty_tile(g),
                                 rhs=basis_y8[:, 512 * h:512 * h + 512],
                                 start=True, stop=True)
            wt = const.tile([128, 1024], bf, tag=f"wy{g}")
            nc.scalar.activation(out=wt[:], in_=pa[:], func=EXP)
            wys.append(wt)

        for g in range(NG):
            wyt = wys[g]
            for jrel in range(8):
                j = 8 * g + jrel
                nc.tensor.matmul(po[:],
                                 lhsT=wyt[:, 128 * jrel:128 * jrel + 128],
                                 rhs=xs[g][:, 128 * jrel:128 * jrel + 128],
                                 start=(j == 0), stop=(j == NCHUNK - 1))

        # ---- normalize: img = num * (1/(den + 1e-8)) ----
        dent = const.tile([128, 32], dt, tag="dent")
        nc.vector.tensor_scalar(out=dent[:], in0=po[:, 96:128], scalar1=1e-8,
                                scalar2=None, op0=add)
        dr = const.tile([128, 32], dt, tag="dr")
        nc.vector.reciprocal_approx_fast(out=dr[:], in_=dent[:])
        img = const.tile([128, 96], dt, tag="img")
        img_r = img[:].rearrange("p (c x) -> p c x", c=3)
        num_r = po[:, 0:96].rearrange("p (c x) -> p c x", c=3)
        dr_b = dr[:].unsqueeze(1).broadcast_to([128, 3, 32])
        nc.vector.tensor_mul(out=img_r, in0=num_r, in1=dr_b)
        nc.sync.dma_start(out=out_d, in_=img[:])

    nc.compile()   # legalizes sync waits (HW allows 1/instruction) etc.
    return nc


def _split3(v, bf):
    """Exact-ish 3-way bf16 split of float64/float32 array v."""
    v = v.astype(F32)
    p1 = v.astype(bf)
    r1 = (v - p1.astype(F32)).astype(F32)
    p2 = r1.astype(bf)
    r2 = (r1 - p2.astype(F32)).astype(F32)
    p3 = r2.astype(bf)
    return p1, p2, p3


def _host_prep(positions, colors, opacities, scales, qvec, tvec):
    """Build the 8 per-core input maps (all O(N) numpy work)."""
    import ml_dtypes
    bf = ml_dtypes.bfloat16

    positions = np.asarray(positions, dtype=np.float64)
    colors = np.asarray(colors, dtype=F32)
    opacities = np.asarray(opacities, dtype=np.float64)
    scales = np.asarray(scales, dtype=np.float64)
    qvec = np.asarray(qvec, dtype=F32)
    tvec = np.asarray(tvec, dtype=F32)

    g_coef = -0.5 / (scales[:, 0] ** 2)                       # [N]
    lnop = np.log(np.maximum(opacities[:, 0], 1e-300))        # [N]

    def basis_rows(q):
        """[16, len(q)] bf16 rows: p2h,p2l,p2h,p2l,p2h,p2l,q,q,q,1,1,1,0*4."""
        q = q.astype(F32)
        p2 = (q * q).astype(F32)
        p2h = p2.astype(bf)
        p2l = (p2 - p2h.astype(F32)).astype(F32).astype(bf)
        qb = q.astype(bf)
        one = np.ones_like(q, dtype=bf)
        zero = np.zeros_like(q, dtype=bf)
        return np.stack([p2h, p2l, p2h, p2l, p2h, p2l,
                         qb, qb, qb, one, one, one, zero, zero, zero, zero])

    def coefT(A, B, C):
        """[N] coefs -> [128, 512] bf16 transposed pack.

        Tile t (cols 128t:128t+128): row 16*jrel + r, col = n_in_chunk,
        rows r: (A1,A1,A2,A2,A3,A3,B1,B2,B3,C1,C2,C3,0,0,0,0)."""
        a1, a2, a3 = _split3(A, bf)
        b1, b2, b3 = _split3(B, bf)
        c1, c2, c3 = _split3(C, bf)
        zero = np.zeros_like(a1)
        rows = np.stack([a1, a1, a2, a2, a3, a3, b1, b2, b3, c1, c2, c3,
                         zero, zero, zero, zero])            # [16, N]
        # [16, 32 chunks, 128 n] -> per tile t: [16*8jrel rows, 128]
        rows = rows.reshape(16, NCHUNK, 128)
        pack = np.zeros((128, 512), bf)
        for j in range(NCHUNK):
            t, jrel = j // 8, j % 8
            pack[16 * jrel:16 * jrel + 16, 128 * t:128 * t + 128] = rows[:, j, :]
        return pack

    # basis_y8 [128, 1024]: block-diag, rows 16*jrel+r, cols 128*jrel + py
    py = np.arange(128) - CY
    by_rows = basis_rows(py)                      # [16, 128]
    basis_y8 = np.zeros((128, 1024), bf)
    for jrel in range(8):
        basis_y8[16 * jrel:16 * jrel + 16, 128 * jrel:128 * jrel + 128] = by_rows

    # car [128, 4096] bf16: col 128*j + 32*c + px = color_c(128j+p) (c=3: 1)
    colc = np.concatenate([colors, np.ones((N, 1), F32)], axis=1)  # [N, 4]
    car = colc.reshape(NCHUNK, 128, 4).transpose(1, 0, 2)          # [128, j, c]
    car = np.repeat(car[:, :, :, None], PXB, axis=3)               # [128,j,c,px]
    car = np.ascontiguousarray(car.reshape(128, NCHUNK * 128)).astype(bf)

    in_maps = []
    for p in range(NPOSE):
        R = _quat2mat(qvec[p])
        t64 = tvec[p].astype(np.float64)
        u = positions @ (FX * R[0]) + FX * t64[0]
        v = positions @ (FY * R[1]) + FY * t64[1]
        zc = positions @ R[2] + t64[2]
        ax = u / zc + CX          # absolute px coords of gaussian center
        ay = v / zc + CY
        ayc = ay - CY             # y centered at 64
        coefT_y = coefT(g_coef, -2.0 * g_coef * ayc, g_coef * ayc * ayc)
        basB = np.zeros((128, 1536), bf)
        basB[:, 0:128] = coefT_y[:, 0:128]
        basB[:, 128:1152] = basis_y8
        basB[:, 1152:1536] = coefT_y[:, 128:512]

        for b in range(NBLK):
            cb = 32.0 * b + 16.0                  # block center
            axc = ax - cb
            coefT_x = coefT(g_coef, -2.0 * g_coef * axc,
                            g_coef * axc * axc + lnop)
            px = np.arange(PXB * b, PXB * b + PXB) - cb   # in [-16, 16)
            bx_rows = basis_rows(px)                      # [16, 32]
            basis_x = np.zeros((128, 256), bf)
            for jrel in range(8):
                basis_x[16 * jrel:16 * jrel + 16,
                        32 * jrel:32 * jrel + 32] = bx_rows
            basA = np.zeros((128, 768), bf)
            basA[:, 0:256] = basis_x
            basA[:, 256:768] = coefT_x
            m = {"basA": basA, "basB": basB}
            for h in range(2):
                m[f"car{h}"] = np.ascontiguousarray(
                    car[:, 2048 * h:2048 * h + 2048])
            in_maps.append(m)
    return in_maps


def _assemble(slabs):
    """slabs: list of 8 [128, 96] arrays -> [NPOSE*16, 3, 32, 32] output."""
    out = []
    for p in range(NPOSE):
        img = np.zeros((H, W, 3), F32)
        for b in range(NBLK):
            slab = slabs[p * NBLK + b]
            for c in range(3):
                img[:, PXB * b:PXB * b + PXB, c] = slab[:, 32 * c:32 * c + 32]
        tiles = img.reshape(H * W, 3).reshape(16, 1024, 3)
        tiles = tiles.transpose(0, 2, 1).reshape(16, 3, 32, 32)
        out.append(tiles)
    return np.concatenate(out, axis=0).astype(F32)


def kernel(positions, colors, opacities, scales, qvec, tvec, _trace=False):
    from concourse.bass_utils import run_bass_kernel_spmd

    if "nc" not in _CACHE:
        _CACHE["nc"] = _build_program()
    nc = _CACHE["nc"]

    in_maps = _host_prep(positions, colors, opacities, scales, qvec, tvec)
    res = run_bass_kernel_spmd(nc, in_maps, core_ids=list(range(8)),
                               trace=_trace)
    slabs = [np.asarray(res.results[c]["out"]) for c in range(8)]
    out = _assemble(slabs)
    if _trace:
        _CACHE["last_result"] = res
    return out


# revision 23
# speedup vs baseline: 1.2252x; 1.0304x over previous
"""Trainium2 Bass kernel for the differentiable gaussian-splat renderer.

Full-input contract: kernel(**inputs) takes the unsharded inputs and returns
the full [2*16, 3, 32, 32] output.

Math (per pose):
    cam = positions @ R.T + t ;  pj = (fx*cam_x/cam_z + cx, fy*cam_y/cam_z + cy)
    w[n, p] = op_n * exp(-0.5*((px-ax_n)^2 + (py-ay_n)^2)/s_n^2)
    img = (w.T @ colors) / (w.T @ 1 + 1e-8)

The gaussian weight is separable: w = wx[n,px] * wy[n,py] (opacity folded into
wx via ln(op) added to the constant coefficient), so instead of N*HW
exponentials we need N*(W + H) and the pixel accumulation becomes a
K=128-chunked matmul  out[py, (c,px)] += wy_chunk.T @ (car_chunk (*) wx_chunk).

Sharding: 8 independent cores = 2 poses x 4 px-column blocks (32 px each).
No collectives; each core computes all 4096 gaussians for its (pose, px-block)
and writes a [128, 96] slab = (py, 32c+px_local). Host reassembles.

All O(N) per-gaussian geometry (projection, quadratic coefficients
[g, -2*g*ax', g*ax'^2 + ln(op)], their exact 3-way bf16 splits, and the
transposed coefficient packs the PE consumes) is precomputed on host; the
device does the O(N*(W+H)) separable evaluation (arg matmuls + exp) and the
O(N*H*W/8) accumulation. The color array is pre-expanded on host to
car[p, (j,c,px)] so the X-build is a contiguous bf16 multiply (DVE fast
path) instead of a slow stride-0-inner broadcast.
"""

import numpy as np

H = 128
W = 128
FX = 120.0
FY = 120.0
CX = 64.0
CY = 64.0
N = 4096
NCHUNK = 32          # 4096 / 128
NPOSE = 2
PXB = 32             # px columns per core
NBLK = 4             # px blocks
NG = 4               # 8-chunk groups
F32 = np.float32

_CACHE = {}


def _quat2mat(q):
    q = np.asarray(q, dtype=np.float64)
    q = q / np.linalg.norm(q)
    w, x, y, z = q
    return np.array([
        [1 - 2 * (y * y + z * z), 2 * (x * y - z * w), 2 * (x * z + y * w)],
        [2 * (x * y + z * w), 1 - 2 * (x * x + z * z), 2 * (y * z - x * w)],
        [2 * (x * z - y * w), 2 * (y * z + x * w), 1 - 2 * (x * x + y * y)],
    ])


def _build_program():
    """Build the SPMD Bass/Tile program (same program on every core)."""
    import concourse.bacc as bacc
    import concourse.tile as tile
    import concourse.mybir as mybir
    from contextlib import ExitStack

    dt = mybir.dt.float32
    bf = mybir.dt.bfloat16
    nc = bacc.Bacc()

    # ---- DRAM I/O (per-core shapes) ----
    # basA (bf16): cols 0:256 basis_x | 256:768 coefT_x (4 tiles of [128,128])
    basA_d = nc.dram_tensor("basA", [128, 768], bf, kind="ExternalInput").ap()
    # basB (bf16): cols 0:128 coefT_y tile0 | 128:1152 basis_y8 |
    #              1152:1536 coefT_y tiles 1-3
    basB_d = nc.dram_tensor("basB", [128, 1536], bf, kind="ExternalInput").ap()
    # car (bf16): col 1024*g + 128*jrel + 32*c + px = color_c(gauss) (c=3: 1.0)
    car_d = [nc.dram_tensor(f"car{h}", [128, 2048], bf, kind="ExternalInput").ap()
             for h in range(2)]
    out_d = nc.dram_tensor("out", [128, 96], dt, kind="ExternalOutput").ap()
    warm_d = nc.dram_tensor("warm", [1, 64], bf, kind="Internal").ap()

    add = mybir.AluOpType.add
    EXP = mybir.ActivationFunctionType.Exp

    with tile.TileContext(nc) as tc, ExitStack() as ctx:
        const = ctx.enter_context(tc.tile_pool(name="const", bufs=1))
        psum_arg = ctx.enter_context(tc.tile_pool(name="psum_arg", bufs=3, space="PSUM"))
        psum_out = ctx.enter_context(tc.tile_pool(name="psum_out", bufs=1, space="PSUM"))

        po = psum_out.tile([128, 128], dt, tag="po")  # claim psum bank 0 first

        basA = const.tile([128, 768], bf, tag="basA")
        nc.sync.dma_start(out=basA[:], in_=basA_d)
        basB = const.tile([128, 1536], bf, tag="basB")
        nc.sync.dma_start(out=basB[:], in_=basB_d)
        car = const.tile([128, 4096], bf, tag="car")
        for h in range(2):
            nc.sync.dma_start(out=car[:, 2048 * h:2048 * h + 2048], in_=car_d[h])

        basis_x = basA[:, 0:256]

        def ctx_tile(t):
            return basA[:, 256 + 128 * t:384 + 128 * t]

        def cty_tile(g):
            return basB[:, 0:128] if g == 0 else basB[:, 1024 + 128 * g:1152 + 128 * g]

        basis_y8 = basB[:, 128:1152]

        # ---- wx args: 4 matmuls [128,128]x[128,256] -> one exp -> bf16 ----
        pa_x = psum_arg.tile([128, 1024], dt, tag="pa")
        for t in range(4):
            nc.tensor.matmul(pa_x[:, 256 * t:256 * t + 256],
                             lhsT=ctx_tile(t),
                             rhs=basis_x, start=True, stop=True)
        w_x = const.tile([128, 1024], bf, tag="wx")
        nc.scalar.activation(out=w_x[:], in_=pa_x[:], func=EXP)

        # ---- X build: per 8-chunk group, X = car (*) wx  (all bf16,
        #      inner-dim contiguous -> DVE fast path) ----
        xs = []
        for g in range(NG):
            Xg = const.tile([128, 1024], bf, tag=f"X{g}")
            car_v = car[:, 1024 * g:1024 * g + 1024].rearrange(
                "p (j c x) -> p j c x", j=8, c=4)
            wx_v = w_x[:, 256 * g:256 * g + 256].rearrange(
                "p (j x) -> p j x", j=8).unsqueeze(2).broadcast_to([128, 8, 4, 32])
            out_v = Xg[:].rearrange("p (j c x) -> p j c x", j=8, c=4)
            nc.vector.tensor_mul(out=out_v, in0=car_v, in1=wx_v)
            xs.append(Xg)

        # ---- wy args (2 matmuls per group) + exp: all emitted upfront so
        #      no acc batch sits between a wy matmul and its exp (the sem
        #      update otherwise lands late and stalls the scalar engine) ----
        wys = []
        for g in range(NG):
            pa = psum_arg.tile([128, 1024], dt, tag="pa")
            for h in range(2):
                nc.tensor.matmul(pa[:, 512 * h:512 * h + 512],
                                 lhsT=cty_tile(g),
                                 rhs=basis_y8[:, 512 * h:512 * h + 512],
                                 start=True, stop=True)
            wt = const.tile([128, 1024], bf, tag=f"wy{g}")
            nc.scalar.activation(out=wt[:], in_=pa[:], func=EXP)
            wys.append(wt)

        # wake the DMA rings ahead of the output transfer
        nc.sync.dma_start(out=warm_d, in_=wys[2][0:1, 0:64])

        for g in range(NG):
            wyt = wys[g]
            for jrel in range(8):
                j = 8 * g + jrel
                nc.tensor.matmul(po[:],
                                 lhsT=wyt[:, 128 * jrel:128 * jrel + 128],
                                 rhs=xs[g][:, 128 * jrel:128 * jrel + 128],
                                 start=(j == 0), stop=(j == NCHUNK - 1))

        # ---- normalize: img = num * (1/(den + 1e-8)) ----
        dent = const.tile([128, 32], dt, tag="dent")
        nc.vector.tensor_scalar(out=dent[:], in0=po[:, 96:128], scalar1=1e-8,
                                scalar2=None, op0=add)
        dr = const.tile([128, 32], dt, tag="dr")
        nc.vector.reciprocal_approx_fast(out=dr[:], in_=dent[:])
        img = const.tile([128, 96], dt, tag="img")
        img_r = img[:].rearrange("p (c x) -> p c x", c=3)
        num_r = po[:, 0:96].rearrange("p (c x) -> p c x", c=3)
        dr_b = dr[:].unsqueeze(1).broadcast_to([128, 3, 32])
        nc.vector.tensor_mul(out=img_r, in0=num_r, in1=dr_b)
        nc.sync.dma_start(out=out_d, in_=img[:])

    nc.compile()   # legalizes sync waits (HW allows 1/instruction) etc.
    return nc


def _split3(v, bf):
    """Exact-ish 3-way bf16 split of float64/float32 array v."""
    v = v.astype(F32)
    p1 = v.astype(bf)
    r1 = (v - p1.astype(F32)).astype(F32)
    p2 = r1.astype(bf)
    r2 = (r1 - p2.astype(F32)).astype(F32)
    p3 = r2.astype(bf)
    return p1, p2, p3


def _host_prep(positions, colors, opacities, scales, qvec, tvec):
    """Build the 8 per-core input maps (all O(N) numpy work)."""
    import ml_dtypes
    bf = ml_dtypes.bfloat16

    positions = np.asarray(positions, dtype=np.float64)
    colors = np.asarray(colors, dtype=F32)
    opacities = np.asarray(opacities, dtype=np.float64)
    scales = np.asarray(scales, dtype=np.float64)
    qvec = np.asarray(qvec, dtype=F32)
    tvec = np.asarray(tvec, dtype=F32)

    g_coef = -0.5 / (scales[:, 0] ** 2)                       # [N]
    lnop = np.log(np.maximum(opacities[:, 0], 1e-300))        # [N]

    def basis_rows(q):
        """[16, len(q)] bf16 rows: p2h,p2l,p2h,p2l,p2h,p2l,q,q,q,1,1,1,0*4."""
        q = q.astype(F32)
        p2 = (q * q).astype(F32)
        p2h = p2.astype(bf)
        p2l = (p2 - p2h.astype(F32)).astype(F32).astype(bf)
        qb = q.astype(bf)
        one = np.ones_like(q, dtype=bf)
        zero = np.zeros_like(q, dtype=bf)
        return np.stack([p2h, p2l, p2h, p2l, p2h, p2l,
                         qb, qb, qb, one, one, one, zero, zero, zero, zero])

    def coefT(A, B, C):
        """[N] coefs -> [128, 512] bf16 transposed pack.

        Tile t (cols 128t:128t+128): row 16*jrel + r, col = n_in_chunk,
        rows r: (A1,A1,A2,A2,A3,A3,B1,B2,B3,C1,C2,C3,0,0,0,0)."""
        a1, a2, a3 = _split3(A, bf)
        b1, b2, b3 = _split3(B, bf)
        c1, c2, c3 = _split3(C, bf)
        zero = np.zeros_like(a1)
        rows = np.stack([a1, a1, a2, a2, a3, a3, b1, b2, b3, c1, c2, c3,
                         zero, zero, zero, zero])            # [16, N]
        # [16, 32 chunks, 128 n] -> per tile t: [16*8jrel rows, 128]
        rows = rows.reshape(16, NCHUNK, 128)
        pack = np.zeros((128, 512), bf)
        for j in range(NCHUNK):
            t, jrel = j // 8, j % 8
            pack[16 * jrel:16 * jrel + 16, 128 * t:128 * t + 128] = rows[:, j, :]
        return pack

    # basis_y8 [128, 1024]: block-diag, rows 16*jrel+r, cols 128*jrel + py
    py = np.arange(128) - CY
    by_rows = basis_rows(py)                      # [16, 128]
    basis_y8 = np.zeros((128, 1024), bf)
    for jrel in range(8):
        basis_y8[16 * jrel:16 * jrel + 16, 128 * jrel:128 * jrel + 128] = by_rows

    # car [128, 4096] bf16: col 128*j + 32*c + px = color_c(128j+p) (c=3: 1)
    colc = np.concatenate([colors, np.ones((N, 1), F32)], axis=1)  # [N, 4]
    car = colc.reshape(NCHUNK, 128, 4).transpose(1, 0, 2)          # [128, j, c]
    car = np.repeat(car[:, :, :, None], PXB, axis=3)               # [128,j,c,px]
    car = np.ascontiguousarray(car.reshape(128, NCHUNK * 128)).astype(bf)

    in_maps = []
    for p in range(NPOSE):
        R = _quat2mat(qvec[p])
        t64 = tvec[p].astype(np.float64)
        u = positions @ (FX * R[0]) + FX * t64[0]
        v = positions @ (FY * R[1]) + FY * t64[1]
        zc = positions @ R[2] + t64[2]
        ax = u / zc + CX          # absolute px coords of gaussian center
        ay = v / zc + CY
        ayc = ay - CY             # y centered at 64
        coefT_y = coefT(g_coef, -2.0 * g_coef * ayc, g_coef * ayc * ayc)
        basB = np.zeros((128, 1536), bf)
        basB[:, 0:128] = coefT_y[:, 0:128]
        basB[:, 128:1152] = basis_y8
        basB[:, 1152:1536] = coefT_y[:, 128:512]

        for b in range(NBLK):
            cb = 32.0 * b + 16.0                  # block center
            axc = ax - cb
            coefT_x = coefT(g_coef, -2.0 * g_coef * axc,
                            g_coef * axc * axc + lnop)
            px = np.arange(PXB * b, PXB * b + PXB) - cb   # in [-16, 16)
            bx_rows = basis_rows(px)                      # [16, 32]
            basis_x = np.zeros((128, 256), bf)
            for jrel in range(8):
                basis_x[16 * jrel:16 * jrel + 16,
                        32 * jrel:32 * jrel + 32] = bx_rows
            basA = np.zeros((128, 768), bf)
            basA[:, 0:256] = basis_x
            basA[:, 256:768] = coefT_x
            m = {"basA": basA, "basB": basB}
            for h in range(2):
                m[f"car{h}"] = np.ascontiguousarray(
                    car[:, 2048 * h:2048 * h + 2048])
            in_maps.append(m)
    return in_maps


def _assemble(slabs):
    """slabs: list of 8 [128, 96] arrays -> [NPOSE*16, 3, 32, 32] output."""
    out = []
    for p in range(NPOSE):
        img = np.zeros((H, W, 3), F32)
        for b in range(NBLK):
            slab = slabs[p * NBLK + b]
            for c in range(3):
                img[:, PXB * b:PXB * b + PXB, c] = slab[:, 32 * c:32 * c + 32]
        tiles = img.reshape(H * W, 3).reshape(16, 1024, 3)
        tiles = tiles.transpose(0, 2, 1).reshape(16, 3, 32, 32)
        out.append(tiles)
    return np.concatenate(out, axis=0).astype(F32)


def kernel(positions, colors, opacities, scales, qvec, tvec, _trace=False):
    from concourse.bass_utils import run_bass_kernel_spmd

    if "nc" not in _CACHE:
        _CACHE["nc"] = _build_program()
    nc = _CACHE["nc"]

    in_maps = _host_prep(positions, colors, opacities, scales, qvec, tvec)
    res = run_bass_kernel_spmd(nc, in_maps, core_ids=list(range(8)),
                               trace=_trace)
    slabs = [np.asarray(res.results[c]["out"]) for c in range(8)]
    out = _assemble(slabs)
    if _trace:
        _CACHE["last_result"] = res
    return out
